# revision 13
# baseline (speedup 1.0000x reference)
"""CircularMemoryBank on 8 trn2 NeuronCores.

Math (D = 4096):
  store:    m[d]   = sum_i sum_j K[i,j] * V[i, (d-j) mod D]
  retrieve: R[q,n] = sum_b Q[q,b] * m[(b+n) mod D]

Both phases are cast as dense PE matmuls, data-parallel over the pair/query
batch axes (512 rows per core):

  store:  with j = 128c + r, accumulate in PSUM over (c, i-chunks):
            H[r, m] = sum_c sum_i K[i, 128c+r] * V[i, (m - 128c) mod D]
          then m[d] = sum_r H[r, (d-r) mod D]  (tiny 128x4096 diagonal sum,
          done host-side together with the cross-core reduction).
  retrieve: R^T[n, q] = sum_b C[b, n] * Q^T[b, q],  C[b,n] = m[(b+n) mod D].
          C tiles come from a host-built sliding-window table Call[p, x] =
          m[(x+p) mod D]; Q^T and the final output transpose are host-side.
"""

import os
import numpy as np
import ml_dtypes

import concourse.bass as bass
import concourse.mybir as mybir
import concourse.tile as tile
from concourse.bass_utils import run_bass_kernel_spmd

D = 4096
NCORES = 8
NS = D // NCORES  # 512 rows per core
BF16 = mybir.dt.bfloat16
F32 = mybir.dt.float32
NPBF16 = ml_dtypes.bfloat16

LAST_EXEC_NS = []  # wall-clock ns per launch

_ws_ctr = [0]


def _split_waits(nc, cap=1):
    """walrus ISA structs hold very few sem-wait slots (1 for Matmult).

    Hoist excess waits from any instruction onto freshly inserted same-engine
    NoOps placed immediately before it, one wait per NoOp.
    """
    for f in nc.m.functions:
        for bb in f.blocks:
            insts = bb.instructions
            out = []
            changed = False
            for ins in insts:
                si = ins.sync_info() if callable(ins.sync_info) else \
                    ins.sync_info
                if si is not None and len(si.on_wait) > cap:
                    waits = list(si.on_wait)
                    for w in waits[:-cap]:
                        nop = mybir.InstNoOp(name=f"ws_{_ws_ctr[0]}")
                        _ws_ctr[0] += 1
                        nop.engine = ins.engine
                        nop.sync_info = mybir.SyncInfo(on_wait=[w],
                                                       on_update=[])
                        out.append(nop)
                    ins.sync_info = mybir.SyncInfo(
                        on_wait=waits[-cap:], on_update=list(si.on_update))
                    changed = True
                out.append(ins)
            if changed:
                bb.instructions = out


def _build_store():
    nc = bass.Bass("TRN2", target_bir_lowering=False, debug=False,
                   num_devices=NCORES)
    k_in = nc.dram_tensor("k_in", [NS, D], BF16, kind="ExternalInput")
    v_in = nc.dram_tensor("v_in", [NS, D], BF16, kind="ExternalInput")
    h_out = nc.dram_tensor("h_out", [128, D], F32, kind="ExternalOutput")

    NI = NS // 128  # 4 i-chunks
    with tile.TileContext(nc) as tc:
        with (
            tc.tile_pool(name="kv", bufs=1) as kv,
            tc.tile_pool(name="hps", bufs=8, space="PSUM") as hps,
            tc.tile_pool(name="hsb", bufs=1) as hsb,
        ):
            h_all = hsb.tile([128, D], F32, name="h_all", tag="hall")
            # one wide tile + one DMA per input => single DMAHW lane each
            k_all = kv.tile([128, NI * D], BF16, name="k_all", tag="ka")
            v_all = kv.tile([128, NI * D], BF16, name="v_all", tag="va")
            nc.sync.dma_start(
                k_all[:].rearrange("p (i j) -> p i j", i=NI),
                k_in.rearrange("(i p) j -> p i j", p=128))
            nc.sync.dma_start(
                v_all[:].rearrange("p (i j) -> p i j", i=NI),
                v_in.rearrange("(i p) j -> p i j", p=128))
            k_sb = [k_all[:, D * i:D * (i + 1)] for i in range(NI)]
            v_sb = [v_all[:, D * i:D * (i + 1)] for i in range(NI)]

            for b in range(8):
                h_ps = hps.tile([128, 512], F32, name=f"h_ps{b}", tag="h")
                for c in range(32):
                    s0 = (512 * b - 128 * c) % D
                    if s0 + 512 <= D:
                        pieces = [(0, s0, 512)]
                    else:
                        ln1 = D - s0
                        pieces = [(0, s0, ln1), (ln1, 0, 512 - ln1)]
                    for i in range(NI):
                        st = (c == 0 and i == 0)
                        sp = (c == 31 and i == NI - 1)
                        for off, src, ln in pieces:
                            nc.tensor.matmul(
                                h_ps[:, off:off + ln],
                                k_sb[i][:, 128 * c:128 * (c + 1)],
                                v_sb[i][:, src:src + ln],
                                start=st, stop=sp,
                            )
                nc.vector.tensor_copy(h_all[:, 512 * b:512 * (b + 1)],
                                      h_ps[:])
            nc.sync.dma_start(h_out[:], h_all[:])
    _split_waits(nc)
    return nc


def _build_retrieve():
    nc = bass.Bass("TRN2", target_bir_lowering=False, debug=False,
                   num_devices=NCORES)
    qt_in = nc.dram_tensor("qt_in", [D, NS], BF16, kind="ExternalInput")
    call_in = nc.dram_tensor("call_in", [128, 8192], BF16,
                             kind="ExternalInput")
    rt_out = nc.dram_tensor("rt_out", [D, NS], F32, kind="ExternalOutput")

    with tile.TileContext(nc) as tc:
        with (
            tc.tile_pool(name="qc", bufs=1) as qc,
            tc.tile_pool(name="rps", bufs=8, space="PSUM") as rps,
            tc.tile_pool(name="rsb", bufs=4) as rsb,
        ):
            call_sb = qc.tile([128, 8192], BF16, name="call_sb", tag="call")
            nc.sync.dma_start(call_sb[:], call_in[:])
            qt_all = qc.tile([128, 32 * NS], BF16, name="qt_all", tag="qa")
            nc.sync.dma_start(
                qt_all[:].rearrange("p (bc q) -> p bc q", bc=32),
                qt_in.rearrange("(bc p) q -> p bc q", p=128))
            qt_sb = [qt_all[:, NS * bc:NS * (bc + 1)] for bc in range(32)]

            for nch in range(32):
                r_ps = rps.tile([128, NS], F32, name=f"r_ps{nch}", tag="r")
                for bc in range(32):
                    t = bc + nch
                    nc.tensor.matmul(
                        r_ps[:],
                        call_sb[:, 128 * t:128 * t + 128],
                        qt_sb[bc][:],
                        start=(bc == 0), stop=(bc == 31),
                    )
                r_sb = rsb.tile([128, NS], F32, name=f"r_sb{nch}", tag="rs")
                if nch % 2 == 0:
                    nc.vector.tensor_copy(r_sb[:], r_ps[:])
                else:
                    nc.scalar.copy(r_sb[:], r_ps[:])
                nc.sync.dma_start(rt_out[128 * nch:128 * (nch + 1), :],
                                  r_sb[:])
    _split_waits(nc)
    return nc


def _run(nc, in_maps):
    import time
    t0 = time.time()
    res = run_bass_kernel_spmd(nc, in_maps, core_ids=list(range(NCORES)))
    LAST_EXEC_NS.append(int((time.time() - t0) * 1e9))
    return res.results


def kernel(keys, values, query_keys):
    keys = np.asarray(keys)
    values = np.asarray(values)
    query_keys = np.asarray(query_keys)

    # ---- store phase: per-core partial H ----
    nc_s = _build_store()
    in_maps = []
    for c in range(NCORES):
        sl = slice(NS * c, NS * (c + 1))
        in_maps.append({
            "k_in": np.ascontiguousarray(keys[sl].astype(NPBF16)),
            "v_in": np.ascontiguousarray(values[sl].astype(NPBF16)),
        })
    outs = _run(nc_s, in_maps)
    h_sum = np.zeros((128, D), np.float32)
    for o in outs:
        h_sum += o["h_out"]

    # m[d] = sum_r H[r, (d-r) mod D]
    idx = (np.arange(D)[None, :] - np.arange(128)[:, None]) % D
    m = h_sum[np.arange(128)[:, None], idx].sum(axis=0)

    # ---- retrieve phase ----
    call = m[(np.arange(8192)[None, :] + np.arange(128)[:, None]) % D]
    call_bf = np.ascontiguousarray(call.astype(NPBF16))
    qt = np.ascontiguousarray(query_keys.T.astype(NPBF16))

    nc_r = _build_retrieve()
    in_maps = []
    for c in range(NCORES):
        in_maps.append({
            "qt_in": np.ascontiguousarray(qt[:, NS * c:NS * (c + 1)]),
            "call_in": call_bf,
        })
    outs = _run(nc_r, in_maps)

    out = np.empty((D, D), np.float32)
    for c in range(NCORES):
        out[NS * c:NS * (c + 1), :] = outs[c]["rt_out"].T
    return out
